# revision 4
# baseline (speedup 1.0000x reference)
"""GNN message-passing kernel (GTEProgramClassification) on 8 Trainium2 cores.

Strategy: dst nodes are partitioned 6250/core (edges are contiguous per dst
since dst_idx is sorted). Host composes the two gathers into one
(cidx = token_id[src_idx]) and marks each segment's last edge with rel=-1 so
the on-device segment sum directly produces child_sum (sum excluding the last
message). Per 128-dst window the device:
  gathers edge rows (indirect DMA) -> builds a one-hot [edge, dst] matrix via
  iota/is_equal -> matmul-accumulates child sums in PSUM -> gathers last-edge
  rows -> transposes via PE -> W matmul + relu(+b) -> ft = last + relu ->
  classifier matmul (+bc) -> writes the [104, 128] output slab.
Outputs are produced transposed [104, nd] per core; the host reassembles.
deg==1 nodes are exact automatically: their only edge is "last" (rel=-1), so
child_sum=0 and ft=last (b is zero per the model spec).
"""
import numpy as np
import concourse.bass as bass
import concourse.bacc as bacc
import concourse.mybir as mybir
import concourse.tile as tile
from concourse.bass_utils import run_bass_kernel_spmd

NCORES = 8
ND = 50000
NDC = ND // NCORES  # 6250
WIN = 128
NW = (NDC + WIN - 1) // WIN  # 49
NDP = NW * WIN  # 6272
V = 50000
D = 256
C = 104
F32 = mybir.dt.float32
I32 = mybir.dt.int32

_cache = {}


def _build(nb):
    nbtot = int(sum(nb))
    nc = bacc.Bacc("TRN2", target_bir_lowering=False, debug=False)
    emb = nc.dram_tensor("emb", [V, D], F32, kind="ExternalInput")
    gidx = nc.dram_tensor("gidx", [128, nbtot], I32, kind="ExternalInput")
    rel = nc.dram_tensor("rel", [128, nbtot], F32, kind="ExternalInput")
    lidx = nc.dram_tensor("lidx", [128, NW], I32, kind="ExternalInput")
    wt = nc.dram_tensor("wt", [128, 2 * D], F32, kind="ExternalInput")
    wc = nc.dram_tensor("wc", [128, 2 * C], F32, kind="ExternalInput")
    b2 = nc.dram_tensor("b2", [128, 2], F32, kind="ExternalInput")
    bc1 = nc.dram_tensor("bc1", [128, 1], F32, kind="ExternalInput")
    iot = nc.dram_tensor("iot", [128, 128], F32, kind="ExternalInput")
    idn = nc.dram_tensor("idn", [128, 128], F32, kind="ExternalInput")
    outT = nc.dram_tensor("outT", [C, NDP], F32, kind="ExternalOutput")

    with tile.TileContext(nc) as tc:
        with (
            tc.tile_pool(name="const", bufs=1) as cpool,
            tc.tile_pool(name="gp", bufs=12) as gpool,
            tc.tile_pool(name="oh", bufs=8) as ohpool,
            tc.tile_pool(name="xp", bufs=2) as xpool,
            tc.tile_pool(name="op", bufs=2) as opool,
            tc.tile_pool(name="ps2", bufs=2, space="PSUM") as psum2,
            tc.tile_pool(name="ps1", bufs=1, space="PSUM") as psum1,
        ):
            def cload(name, src, shape, dt):
                t = cpool.tile(shape, dt, tag=name)
                nc.gpsimd.dma_start(out=t[:], in_=src[:, :])
                return t

            gidx_sb = cload("gidx", gidx, [128, nbtot], I32)
            rel_sb = cload("rel", rel, [128, nbtot], F32)
            lidx_sb = cload("lidx", lidx, [128, NW], I32)
            wt_sb = cload("wt", wt, [128, 2 * D], F32)
            wc_sb = cload("wc", wc, [128, 2 * C], F32)
            b2_sb = cload("b2", b2, [128, 2], F32)
            bc_sb = cload("bc", bc1, [128, 1], F32)
            iota_sb = cload("iot", iot, [128, 128], F32)
            id_sb = cload("idn", idn, [128, 128], F32)

            b = 0
            for w in range(NW):
                nbw = int(nb[w])
                child_ps = psum2.tile([128, D], F32, tag="child")
                last_sb = gpool.tile([128, D], F32, tag="last")
                nc.gpsimd.indirect_dma_start(
                    out=last_sb[:], out_offset=None, in_=emb[:, :],
                    in_offset=bass.IndirectOffsetOnAxis(
                        ap=lidx_sb[:, w : w + 1], axis=0),
                )
                for j in range(nbw):
                    msgs = gpool.tile([128, D], F32, tag="msgs")
                    nc.gpsimd.indirect_dma_start(
                        out=msgs[:], out_offset=None, in_=emb[:, :],
                        in_offset=bass.IndirectOffsetOnAxis(
                            ap=gidx_sb[:, b : b + 1], axis=0),
                    )
                    oh = ohpool.tile([128, 128], F32, tag="oh")
                    nc.vector.tensor_scalar(
                        oh[:], iota_sb[:], rel_sb[:, b : b + 1], None,
                        mybir.AluOpType.is_equal,
                    )
                    nc.tensor.matmul(
                        out=child_ps[:], lhsT=oh[:], rhs=msgs[:],
                        start=(j == 0), stop=(j == nbw - 1),
                    )
                    b += 1
                X = xpool.tile([128, D], F32, tag="X")
                nc.vector.tensor_copy(out=X[:], in_=child_ps[:])
                xt_ps = psum2.tile([128, D], F32, tag="xt")
                for kc in range(2):
                    nc.tensor.transpose(
                        out=xt_ps[:, kc * 128 : (kc + 1) * 128],
                        in_=X[:, kc * 128 : (kc + 1) * 128], identity=id_sb[:])
                xt_sb = xpool.tile([128, D], F32, tag="xts")
                nc.vector.tensor_copy(out=xt_sb[:], in_=xt_ps[:])
                ht_ps = psum1.tile([128, D], F32, tag="ht")
                for jh in range(2):
                    for kc in range(2):
                        nc.tensor.matmul(
                            out=ht_ps[:, jh * 128 : (jh + 1) * 128],
                            lhsT=wt_sb[:, kc * D + jh * 128 : kc * D + (jh + 1) * 128],
                            rhs=xt_sb[:, kc * 128 : (kc + 1) * 128],
                            start=(kc == 0), stop=(kc == 1),
                        )
                rt_sb = xpool.tile([128, D], F32, tag="rt")
                for jh in range(2):
                    nc.scalar.activation(
                        out=rt_sb[:, jh * 128 : (jh + 1) * 128],
                        in_=ht_ps[:, jh * 128 : (jh + 1) * 128],
                        func=mybir.ActivationFunctionType.Relu,
                        bias=b2_sb[:, jh : jh + 1],
                    )
                lt_ps = psum1.tile([128, D], F32, tag="lt")
                for kc in range(2):
                    nc.tensor.transpose(
                        out=lt_ps[:, kc * 128 : (kc + 1) * 128],
                        in_=last_sb[:, kc * 128 : (kc + 1) * 128], identity=id_sb[:])
                ft_sb = xpool.tile([128, D], F32, tag="ft")
                nc.vector.tensor_add(out=ft_sb[:], in0=lt_ps[:], in1=rt_sb[:])
                o_ps = psum1.tile([C, 128], F32, tag="ops")
                for kc in range(2):
                    nc.tensor.matmul(
                        out=o_ps[:], lhsT=wc_sb[:, kc * C : (kc + 1) * C],
                        rhs=ft_sb[:, kc * 128 : (kc + 1) * 128],
                        start=(kc == 0), stop=(kc == 1),
                    )
                o_sb = opool.tile([C, 128], F32, tag="osb")
                nc.vector.tensor_scalar_add(o_sb[:], o_ps[:], bc_sb[:C, :1])
                nc.gpsimd.dma_start(out=outT[:, w * 128 : (w + 1) * 128], in_=o_sb[:])
    nc.compile()
    return nc


def _prep(emb, W, b, Wc, bc, token_id, src_idx, dst_idx):
    E = src_idx.shape[0]
    cidx = token_id[src_idx].astype(np.int32)
    deg = np.bincount(dst_idx, minlength=ND)
    ends = np.cumsum(deg)
    starts = ends - deg
    lidx_all = cidx[ends - 1]
    is_last = np.zeros(E, dtype=bool)
    is_last[ends - 1] = True
    rel_all = ((dst_idx % NDC) % WIN).astype(np.float32)
    rel_all[is_last] = -1.0

    # per (core, window) edge ranges and block counts
    es = np.empty((NCORES, NW), dtype=np.int64)
    ee = np.empty((NCORES, NW), dtype=np.int64)
    for c in range(NCORES):
        for w in range(NW):
            dlo = c * NDC + w * WIN
            dhi = min(c * NDC + (w + 1) * WIN, (c + 1) * NDC)
            es[c, w] = starts[dlo]
            ee[c, w] = ends[dhi - 1]
    cnt = ee - es
    nb = np.maximum(1, (cnt.max(axis=0) + 127) // 128)  # uniform across cores
    nbtot = int(nb.sum())

    in_maps = []
    wth = np.zeros((128, 2 * D), dtype=np.float32)
    for kc in range(2):
        wth[:, kc * D : (kc + 1) * D] = W[:, kc * 128 : (kc + 1) * 128].T
    wch = np.zeros((128, 2 * C), dtype=np.float32)
    for kc in range(2):
        wch[:, kc * C : (kc + 1) * C] = Wc[:, kc * 128 : (kc + 1) * 128].T
    b2h = np.ascontiguousarray(b.reshape(2, 128).T.astype(np.float32))
    bch = np.zeros((128, 1), dtype=np.float32)
    bch[:C, 0] = bc
    iota_h = np.tile(np.arange(128, dtype=np.float32), (128, 1))
    idn_h = np.eye(128, dtype=np.float32)

    for c in range(NCORES):
        gidx_a = np.zeros((nbtot * 128,), dtype=np.int32)
        rel_a = np.full((nbtot * 128,), -1.0, dtype=np.float32)
        off = 0
        for w in range(NW):
            n = int(cnt[c, w])
            seg = slice(es[c, w], ee[c, w])
            gidx_a[off : off + n] = cidx[seg]
            rel_a[off : off + n] = rel_all[seg]
            off += int(nb[w]) * 128
        lid = np.zeros((NDP,), dtype=np.int32)
        lid[:NDC] = lidx_all[c * NDC : (c + 1) * NDC]
        in_maps.append({
            "emb": emb,
            "gidx": np.ascontiguousarray(gidx_a.reshape(nbtot, 128).T),
            "rel": np.ascontiguousarray(rel_a.reshape(nbtot, 128).T),
            "lidx": np.ascontiguousarray(lid.reshape(NW, 128).T),
            "wt": wth, "wc": wch, "b2": b2h, "bc1": bch,
            "iot": iota_h, "idn": idn_h,
        })
    return tuple(nb.tolist()), in_maps


def kernel(emb, W, b, Wc, bc, token_id, src_idx, dst_idx):
    emb = np.asarray(emb, dtype=np.float32)
    W = np.asarray(W, dtype=np.float32)
    b = np.asarray(b, dtype=np.float32)
    Wc = np.asarray(Wc, dtype=np.float32)
    bc = np.asarray(bc, dtype=np.float32)
    token_id = np.asarray(token_id, dtype=np.int32)
    src_idx = np.asarray(src_idx, dtype=np.int32)
    dst_idx = np.asarray(dst_idx, dtype=np.int32)

    nb, in_maps = _prep(emb, W, b, Wc, bc, token_id, src_idx, dst_idx)
    if nb not in _cache:
        _cache[nb] = _build(list(nb))
    nc = _cache[nb]
    res = run_bass_kernel_spmd(nc, in_maps, core_ids=list(range(NCORES)))
    out = np.concatenate(
        [res.results[c]["outT"][:, :NDC].T for c in range(NCORES)], axis=0
    )
    return np.ascontiguousarray(out, dtype=np.float32)
